# revision 1
# baseline (speedup 1.0000x reference)
"""Conservative CAM kernel variant: no xbar DMA-transpose, no DRAM bounce.
xfT is built with 128 PE transposes per sample (plain matmul-mode ops only).
Slower (~PE +25us/sample) but uses only ubiquitously-tested primitives.
Same host-side contract as kernel.py.
"""
import numpy as np

import concourse.mybir as mybir
import concourse.tile as tile
from concourse import bacc
from concourse.bass_utils import run_bass_kernel_spmd
from concourse.masks import make_identity

B, C, HW = 16, 512, 64 * 64
N_CORES = 8
BPC = B // N_CORES

F32 = mybir.dt.float32
BF16 = mybir.dt.bfloat16
AF = mybir.ActivationFunctionType

NI = C // 128
NK = HW // 128
NN = HW // 512


def _build_sample(tc, pools, x, out, gam, ident, ident16, s):
    nc = tc.nc
    (p_xf32, p_xf16, p_xfT, p_E, p_ET, p_stage, p_small, p_ps_e, p_ps_t, p_ps_m) = pools

    xf32 = []
    for i in range(NI):
        xt = p_xf32.tile([128, HW], F32, tag="xf32", name=f"xf32_{s}_{i}")
        nc.sync.dma_start(xt[:], x[s, 128 * i : 128 * (i + 1), :])
        xf32.append(xt)

    xf16 = []
    for i in range(NI):
        c16 = p_xf16.tile([128, HW], BF16, tag="xf16", name=f"xf16_{s}_{i}")
        nc.vector.tensor_copy(c16[:], xf32[i][:])
        xf16.append(c16)

    # xfT via PE transposes of 128x128 fp32 blocks; the PSUM->SBUF copy
    # casts to bf16. Copies alternate between DVE and ACT to split load.
    xfT = p_xfT.tile([128, NK, C], BF16, tag="xfT", name=f"xfT_{s}")
    for i in range(NI):
        for k in range(NK):
            ps_x = p_ps_t.tile(
                [128, 128], F32, tag="ps_t", name=f"ps_x_{s}_{i}_{k}"
            )
            nc.tensor.transpose(
                ps_x[:], xf32[i][:, 128 * k : 128 * (k + 1)], ident[:]
            )
            dst = xfT[:, k, 128 * i : 128 * (i + 1)]
            if k % 3 == 0:
                nc.vector.tensor_copy(dst, ps_x[:])
            else:
                nc.scalar.copy(dst, ps_x[:])

    Es, scales = [], []
    for j in range(NI):
        ps_e = p_ps_e.tile([128, C], F32, tag="ps_e", name=f"ps_e_{s}_{j}")
        for k in range(NK):
            nc.tensor.matmul(
                ps_e[:],
                lhsT=xfT[:, k, 128 * j : 128 * (j + 1)],
                rhs=xfT[:, k, :],
                start=(k == 0),
                stop=(k == NK - 1),
            )
        negmax = p_small.tile([128, 1], F32, tag="small", name=f"negmax_{s}_{j}")
        nc.vector.reduce_max(negmax[:], ps_e[:], axis=mybir.AxisListType.X)
        nc.vector.tensor_scalar_mul(negmax[:], negmax[:], -1.0)
        Ej = p_E.tile([128, C], F32, tag="E", name=f"E_{s}_{j}")
        ssum = p_small.tile([128, 1], F32, tag="small", name=f"ssum_{s}_{j}")
        nc.scalar.activation(
            Ej[:], ps_e[:], AF.Exp, bias=negmax[:], scale=1.0, accum_out=ssum[:]
        )
        sc = p_small.tile([128, 1], F32, tag="small", name=f"scale_{s}_{j}")
        nc.vector.reciprocal(sc[:], ssum[:])
        nc.vector.tensor_mul(sc[:], sc[:], gam[:])
        Es.append(Ej)
        scales.append(sc)

    ETs = []
    for dd in range(NI):
        ET = p_ET.tile([128, C], BF16, tag="ET", name=f"ET_{s}_{dd}")
        for j in range(NI):
            ps_t = p_ps_t.tile([128, 128], F32, tag="ps_t", name=f"ps_t_{s}_{dd}_{j}")
            nc.tensor.transpose(ps_t[:], Es[j][:, 128 * dd : 128 * (dd + 1)], ident[:])
            nc.scalar.copy(ET[:, 128 * j : 128 * (j + 1)], ps_t[:])
        ETs.append(ET)

    # out-matmul; N=512 is the PSUM single-bank ceiling per matmul.
    NW = 512
    for j in range(NI):
        for nn in range(HW // NW):
            ps_m = p_ps_m.tile([128, NW], F32, tag="ps_m", name=f"ps_m_{s}_{nn}_{j}")
            for dd in range(NI):
                nc.tensor.matmul(
                    ps_m[:],
                    lhsT=ETs[dd][:, 128 * j : 128 * (j + 1)],
                    rhs=xf16[dd][:, NW * nn : NW * (nn + 1)],
                    start=(dd == 0),
                    stop=(dd == NI - 1),
                )
            stg = p_stage.tile([128, NW], F32, tag="stage", name=f"stg_{s}_{nn}_{j}")
            nc.scalar.activation(stg[:], ps_m[:], AF.Copy, scale=scales[j][:])
            nc.vector.tensor_add(
                stg[:], stg[:], xf32[j][:, NW * nn : NW * (nn + 1)]
            )
            nc.sync.dma_start(
                out=out[s, 128 * j : 128 * (j + 1), NW * nn : NW * (nn + 1)],
                in_=stg[:],
            )


def build_program():
    nc = bacc.Bacc("TRN2", target_bir_lowering=False, debug=False, num_devices=N_CORES)
    x = nc.dram_tensor("x", [BPC, C, HW], F32, kind="ExternalInput").ap()
    gamma = nc.dram_tensor("gamma", [128, 1], F32, kind="ExternalInput").ap()
    out = nc.dram_tensor("out", [BPC, C, HW], F32, kind="ExternalOutput").ap()

    with tile.TileContext(nc) as tc:
        with (
            tc.tile_pool(name="const", bufs=1) as p_const,
            tc.tile_pool(name="xf32", bufs=5) as p_xf32,
            tc.tile_pool(name="xf16", bufs=5) as p_xf16,
            tc.tile_pool(name="xfT", bufs=1) as p_xfT,
            tc.tile_pool(name="E", bufs=5) as p_E,
            tc.tile_pool(name="ET", bufs=5) as p_ET,
            tc.tile_pool(name="stage", bufs=6) as p_stage,
            tc.tile_pool(name="small", bufs=16) as p_small,
            tc.tile_pool(name="ps_e", bufs=2, space="PSUM") as p_ps_e,
            tc.tile_pool(name="ps_t", bufs=4, space="PSUM") as p_ps_t,
            tc.tile_pool(name="ps_m", bufs=2, space="PSUM") as p_ps_m,
        ):
            ident = p_const.tile([128, 128], F32)
            make_identity(nc, ident[:])
            ident16 = None
            gam = p_const.tile([128, 1], F32)
            nc.sync.dma_start(gam[:], gamma[:])

            pools = (p_xf32, p_xf16, p_xfT, p_E, p_ET, p_stage, p_small,
                     p_ps_e, p_ps_t, p_ps_m)
            for s in range(BPC):
                _build_sample(tc, pools, x, out, gam, ident, ident16, s)
    nc.compile()
    return nc


_CACHED_NC = None


def kernel(x: np.ndarray, gamma: np.ndarray) -> np.ndarray:
    global _CACHED_NC
    x = np.asarray(x, dtype=np.float32)
    gamma = np.asarray(gamma, dtype=np.float32)
    assert x.shape == (B, C, 64, 64), x.shape
    if _CACHED_NC is None:
        _CACHED_NC = build_program()
    nc = _CACHED_NC

    xr = np.ascontiguousarray(x.reshape(B, C, HW))
    gb = np.full((128, 1), np.asarray(gamma).reshape(-1)[0], dtype=np.float32)
    in_maps = [
        {"x": xr[BPC * c : BPC * (c + 1)], "gamma": gb} for c in range(N_CORES)
    ]
    res = run_bass_kernel_spmd(nc, in_maps, core_ids=list(range(N_CORES)))
    out = np.concatenate([res.results[c]["out"] for c in range(N_CORES)], axis=0)
    return out.reshape(B, C, 64, 64).astype(np.float32)



# revision 2
# speedup vs baseline: 7.2559x; 7.2559x over previous
"""Conservative CAM kernel variant: no xbar DMA-transpose, no DRAM bounce.
xfT is built with 128 PE transposes per sample (plain matmul-mode ops only).
Slower (~PE +25us/sample) but uses only ubiquitously-tested primitives.
Same host-side contract as kernel.py.
"""
import numpy as np

import concourse.mybir as mybir
import concourse.tile as tile
from concourse import bacc
from concourse.bass_utils import run_bass_kernel_spmd
from concourse.masks import make_identity

B, C, HW = 16, 512, 64 * 64
N_CORES = 8
BPC = B // N_CORES

F32 = mybir.dt.float32
BF16 = mybir.dt.bfloat16
AF = mybir.ActivationFunctionType

NI = C // 128
NK = HW // 128
NN = HW // 512


def _build_sample(tc, pools, x, out, gam, ident, ident16, s):
    nc = tc.nc
    (p_xf32, p_xf16, p_xfT, p_E, p_ET, p_stage, p_small, p_ps_e, p_ps_t, p_ps_m) = pools

    xf32 = []
    for i in range(NI):
        xt = p_xf32.tile([128, HW], F32, tag="xf32", name=f"xf32_{s}_{i}")
        nc.sync.dma_start(xt[:], x[s, 128 * i : 128 * (i + 1), :])
        xf32.append(xt)

    xf16 = []
    for i in range(NI):
        c16 = p_xf16.tile([128, HW], BF16, tag="xf16", name=f"xf16_{s}_{i}")
        nc.vector.tensor_copy(c16[:], xf32[i][:])
        xf16.append(c16)

    # xfT via PE transposes of 128x128 fp32 blocks; the PSUM->SBUF copy
    # casts to bf16. Copies alternate between DVE and ACT to split load.
    xfT = p_xfT.tile([128, NK, C], BF16, tag="xfT", name=f"xfT_{s}")
    for i in range(NI):
        for k in range(NK):
            ps_x = p_ps_t.tile(
                [128, 128], F32, tag="ps_t", name=f"ps_x_{s}_{i}_{k}"
            )
            nc.tensor.transpose(
                ps_x[:], xf32[i][:, 128 * k : 128 * (k + 1)], ident[:]
            )
            dst = xfT[:, k, 128 * i : 128 * (i + 1)]
            if k % 3 == 0:
                nc.vector.tensor_copy(dst, ps_x[:])
            else:
                nc.scalar.copy(dst, ps_x[:])

    Es, scales = [], []
    for j in range(NI):
        ps_e = p_ps_e.tile([128, C], F32, tag="ps_e", name=f"ps_e_{s}_{j}")
        for k in range(NK):
            nc.tensor.matmul(
                ps_e[:],
                lhsT=xfT[:, k, 128 * j : 128 * (j + 1)],
                rhs=xfT[:, k, :],
                start=(k == 0),
                stop=(k == NK - 1),
            )
        negmax = p_small.tile([128, 1], F32, tag="small", name=f"negmax_{s}_{j}")
        nc.vector.reduce_max(negmax[:], ps_e[:], axis=mybir.AxisListType.X)
        nc.vector.tensor_scalar_mul(negmax[:], negmax[:], -1.0)
        Ej = p_E.tile([128, C], F32, tag="E", name=f"E_{s}_{j}")
        ssum = p_small.tile([128, 1], F32, tag="small", name=f"ssum_{s}_{j}")
        nc.scalar.activation(
            Ej[:], ps_e[:], AF.Exp, bias=negmax[:], scale=1.0, accum_out=ssum[:]
        )
        sc = p_small.tile([128, 1], F32, tag="small", name=f"scale_{s}_{j}")
        nc.vector.reciprocal(sc[:], ssum[:])
        nc.vector.tensor_mul(sc[:], sc[:], gam[:])
        Es.append(Ej)
        scales.append(sc)

    ETs = []
    for dd in range(NI):
        ET = p_ET.tile([128, C], BF16, tag="ET", name=f"ET_{s}_{dd}")
        for j in range(NI):
            ps_t = p_ps_t.tile([128, 128], F32, tag="ps_t", name=f"ps_t_{s}_{dd}_{j}")
            nc.tensor.transpose(ps_t[:], Es[j][:, 128 * dd : 128 * (dd + 1)], ident[:])
            nc.scalar.copy(ET[:, 128 * j : 128 * (j + 1)], ps_t[:])
        ETs.append(ET)

    # out-matmul; N=512 is the PSUM single-bank ceiling per matmul.
    NW = 512
    for j in range(NI):
        for nn in range(HW // NW):
            ps_m = p_ps_m.tile([128, NW], F32, tag="ps_m", name=f"ps_m_{s}_{nn}_{j}")
            for dd in range(NI):
                nc.tensor.matmul(
                    ps_m[:],
                    lhsT=ETs[dd][:, 128 * j : 128 * (j + 1)],
                    rhs=xf16[dd][:, NW * nn : NW * (nn + 1)],
                    start=(dd == 0),
                    stop=(dd == NI - 1),
                )
            stg = p_stage.tile([128, NW], F32, tag="stage", name=f"stg_{s}_{nn}_{j}")
            nc.scalar.activation(stg[:], ps_m[:], AF.Copy, scale=scales[j][:])
            nc.vector.tensor_add(
                stg[:], stg[:], xf32[j][:, NW * nn : NW * (nn + 1)]
            )
            nc.sync.dma_start(
                out=out[s, 128 * j : 128 * (j + 1), NW * nn : NW * (nn + 1)],
                in_=stg[:],
            )


def build_program():
    nc = bacc.Bacc("TRN2", target_bir_lowering=False, debug=False, num_devices=N_CORES)
    x = nc.dram_tensor("x", [BPC, C, HW], F32, kind="ExternalInput").ap()
    gamma = nc.dram_tensor("gamma", [128, 1], F32, kind="ExternalInput").ap()
    out = nc.dram_tensor("out", [BPC, C, HW], F32, kind="ExternalOutput").ap()

    with tile.TileContext(nc) as tc:
        with (
            tc.tile_pool(name="const", bufs=1) as p_const,
            tc.tile_pool(name="xf32", bufs=5) as p_xf32,
            tc.tile_pool(name="xf16", bufs=5) as p_xf16,
            tc.tile_pool(name="xfT", bufs=1) as p_xfT,
            tc.tile_pool(name="E", bufs=5) as p_E,
            tc.tile_pool(name="ET", bufs=5) as p_ET,
            tc.tile_pool(name="stage", bufs=6) as p_stage,
            tc.tile_pool(name="small", bufs=16) as p_small,
            tc.tile_pool(name="ps_e", bufs=2, space="PSUM") as p_ps_e,
            tc.tile_pool(name="ps_t", bufs=4, space="PSUM") as p_ps_t,
            tc.tile_pool(name="ps_m", bufs=2, space="PSUM") as p_ps_m,
        ):
            ident = p_const.tile([128, 128], F32)
            make_identity(nc, ident[:])
            ident16 = None
            gam = p_const.tile([128, 1], F32)
            nc.sync.dma_start(gam[:], gamma[:])

            pools = (p_xf32, p_xf16, p_xfT, p_E, p_ET, p_stage, p_small,
                     p_ps_e, p_ps_t, p_ps_m)
            for s in range(BPC):
                _build_sample(tc, pools, x, out, gam, ident, ident16, s)
    nc.compile()
    return nc


_CACHED_NC = None


def shard_inputs(x, gamma):
    xr = np.ascontiguousarray(np.asarray(x, np.float32).reshape(B, C, HW))
    gb = np.full((128, 1), np.asarray(gamma).reshape(-1)[0], dtype=np.float32)
    return [
        {"x": xr[BPC * c : BPC * (c + 1)], "gamma": gb} for c in range(N_CORES)
    ]


def unshard_output(res_out):
    """res_out: [N_CORES, BPC, C, HW] (stacked per-core 'out' tensors)."""
    return np.asarray(res_out).reshape(B, C, 64, 64).astype(np.float32)


def kernel(x: np.ndarray, gamma: np.ndarray) -> np.ndarray:
    global _CACHED_NC
    x = np.asarray(x, dtype=np.float32)
    gamma = np.asarray(gamma, dtype=np.float32)
    assert x.shape == (B, C, 64, 64), x.shape
    if _CACHED_NC is None:
        _CACHED_NC = build_program()
    nc = _CACHED_NC

    in_maps = shard_inputs(x, gamma)
    res = run_bass_kernel_spmd(nc, in_maps, core_ids=list(range(N_CORES)))
    out = np.stack([res.results[c]["out"] for c in range(N_CORES)], axis=0)
    return unshard_output(out)

